# revision 1
# baseline (speedup 1.0000x reference)
"""Multi-head attention (B=4, L=2048, D=512, H=8) on 8 Trainium2 cores.

Sharding: core c handles batch b = c//2, query rows [(c%2)*1024, +1024).
K/V projections are split across the two cores sharing a batch (each
projects its own 1024-token half) and exchanged with a pairwise
AllGather through shared DRAM, so attention is fully local afterward.

Device layouts (per core):
  xqT/xkT/xvT (512, 1024)  input slices, transposed (dmodel on partitions)
  qT_all / kT_all          projections kept transposed: head h lives in
                           dmodel-chunk tile h//2 at partition offset 64*(h%2)
  V_sb (128, 520) x16      V natural layout per kv chunk; head h at cols
                           [65h, 65h+64), col 65h+64 = ones (softmax denom)
  scoresT (128kv, 1024q)   PSUM; exp+mask+scale fused into one ACT op
  xsT_ext (65, 512)        PSUM, row 64 = softmax denominator
"""
import numpy as np
import ml_dtypes

import concourse.bacc as bacc
import concourse.bass as bass
import concourse.mybir as mybir
import concourse.tile as tile
from concourse.bass_utils import run_bass_kernel_spmd

F32 = mybir.dt.float32
BF16 = mybir.dt.bfloat16
AF = mybir.ActivationFunctionType

B, L, D = 4, 2048, 512
H, DK = 8, 64
N_CORES = 8
LQ = L // 2            # query rows per core / kv rows projected per core
P = 128
KVC = L // P           # 16 kv chunks
QT = LQ // P           # 8 query tiles of 128
MC = D // P            # 4 dmodel chunks
MASK_BIAS = np.float32(-1e30)

MM_DT = BF16
MM_NP = ml_dtypes.bfloat16 if MM_DT == BF16 else np.float32

_cache = {}


def _build():
    nc = bacc.Bacc("TRN2", target_bir_lowering=False, debug=False,
                   num_devices=N_CORES)

    xqT_d = nc.dram_tensor("xqT", [D, LQ], MM_DT, kind="ExternalInput").ap()
    xkT_d = nc.dram_tensor("xkT", [D, LQ], MM_DT, kind="ExternalInput").ap()
    xvT_d = nc.dram_tensor("xvT", [D, LQ], MM_DT, kind="ExternalInput").ap()
    wq_d = nc.dram_tensor("wq", [D, D], MM_DT, kind="ExternalInput").ap()
    wk_d = nc.dram_tensor("wk", [D, D], MM_DT, kind="ExternalInput").ap()
    wv_d = nc.dram_tensor("wv", [D, D], MM_DT, kind="ExternalInput").ap()
    wo_d = nc.dram_tensor("wo", [D, D], MM_DT, kind="ExternalInput").ap()
    bq_d = nc.dram_tensor("bq", [P, MC], F32, kind="ExternalInput").ap()
    bk_d = nc.dram_tensor("bk", [P, MC], F32, kind="ExternalInput").ap()
    bv_d = nc.dram_tensor("bv", [1, D], MM_DT, kind="ExternalInput").ap()
    bo_d = nc.dram_tensor("bo", [1, D], F32, kind="ExternalInput").ap()
    mb_d = nc.dram_tensor("mb", [P, KVC], F32, kind="ExternalInput").ap()
    out_d = nc.dram_tensor("out", [LQ, D], F32, kind="ExternalOutput").ap()

    PAIRS = [[2 * i, 2 * i + 1] for i in range(N_CORES // 2)]

    with tile.TileContext(nc) as tc:
        with tc.tile_pool(name="const", bufs=1) as cpool, \
             tc.tile_pool(name="xin", bufs=1) as xpool, \
             tc.tile_pool(name="proj", bufs=1) as prpool, \
             tc.tile_pool(name="attn", bufs=17) as apool, \
             tc.tile_pool(name="norm", bufs=4) as npool, \
             tc.tile_pool(name="outp", bufs=3) as opool, \
             tc.tile_pool(name="dram", bufs=1, space="DRAM") as dpool, \
             tc.tile_pool(name="ps", bufs=2, space="PSUM") as ps:

            def load_chunks(pool, ap2d, nm):
                out = []
                for kc in range(MC):
                    t = pool.tile([P, ap2d.shape[1]], ap2d.dtype,
                                  tag=f"{nm}{kc}", name=f"{nm}{kc}")
                    nc.sync.dma_start(t[:], ap2d[kc * P:(kc + 1) * P, :])
                    out.append(t)
                return out

            # interleave weight/input chunk loads in first-use order
            wq = load_chunks(cpool, wq_d, "wq")
            xqT = load_chunks(xpool, xqT_d, "xq")
            bq = cpool.tile_from(bq_d)
            wk = load_chunks(cpool, wk_d, "wk")
            xkT = load_chunks(xpool, xkT_d, "xk")
            bk = cpool.tile_from(bk_d)
            wv = load_chunks(cpool, wv_d, "wv")
            xvT = load_chunks(xpool, xvT_d, "xv")
            wo = load_chunks(cpool, wo_d, "wo")
            bv = cpool.tile_from(bv_d)
            bo = cpool.tile_from(bo_d)
            mb = cpool.tile_from(mb_d)
            ones1 = cpool.tile([1, P], MM_DT)
            nc.vector.memset(ones1[:], 1.0)
            bo_bc = cpool.tile([P, D], F32)
            nc.gpsimd.partition_broadcast(bo_bc[:], bo[:])

            # collective exchange buffers (pairwise AllGather of K/V halves)
            k_own_d = dpool.tile([MC, P, LQ], MM_DT)
            v_own_d = dpool.tile([KVC // 2, P, H * 65], MM_DT)
            k_all_d = dpool.tile([2, MC, P, LQ], MM_DT)
            v_all_d = dpool.tile([2, KVC // 2, P, H * 65], MM_DT)

            # ---- Q projection + own-half K projection (transposed) ----
            qT = [prpool.tile([P, LQ], MM_DT, tag=f"qT{m}", name=f"qT{m}")
                  for m in range(MC)]
            kTo = [prpool.tile([P, LQ], MM_DT, tag=f"kTo{m}", name=f"kTo{m}")
                   for m in range(MC)]
            for m in range(MC):
                for s in range(LQ // 512):
                    pp = ps.tile([P, 512], F32, tag="proj")
                    for kc in range(MC):
                        nc.tensor.matmul(
                            pp[:], wq[kc][:, m * P:(m + 1) * P],
                            xqT[kc][:, s * 512:(s + 1) * 512],
                            start=kc == 0, stop=kc == MC - 1)
                    nc.vector.tensor_scalar_add(qT[m][:, s * 512:(s + 1) * 512],
                                                pp[:], bq[:, m:m + 1])
                for s in range(LQ // 512):
                    pp = ps.tile([P, 512], F32, tag="proj")
                    for kc in range(MC):
                        nc.tensor.matmul(
                            pp[:], wk[kc][:, m * P:(m + 1) * P],
                            xkT[kc][:, s * 512:(s + 1) * 512],
                            start=kc == 0, stop=kc == MC - 1)
                    nc.vector.tensor_scalar_add(kTo[m][:, s * 512:(s + 1) * 512],
                                                pp[:], bk[:, m:m + 1])
                nc.sync.dma_start(k_own_d[m], kTo[m][:])

            # ---- own-half V projection (natural layout + ones columns) ----
            Vo = [prpool.tile([P, H * 65], MM_DT, tag=f"Vo{t}", name=f"Vo{t}")
                  for t in range(KVC // 2)]
            for t in range(KVC // 2):
                pv = ps.tile([P, D], F32, tag="proj")
                for kc in range(MC):
                    nc.tensor.matmul(pv[:], xvT[kc][:, t * P:(t + 1) * P],
                                     wv[kc][:, :], start=kc == 0, stop=False)
                nc.tensor.matmul(pv[:], ones1[0:1, :], bv[0:1, :],
                                 start=False, stop=True)
                vv = Vo[t].rearrange("p (g d) -> p g d", d=65)
                nc.vector.tensor_copy(vv[:, :, 0:64],
                                      pv.rearrange("p (g d) -> p g d", d=64))
                nc.vector.memset(vv[:, :, 64:65], 1.0)
                nc.sync.dma_start(v_own_d[t], Vo[t][:])

            # ---- pairwise K/V exchange ----
            nc.gpsimd.collective_compute(
                "AllGather", mybir.AluOpType.bypass, replica_groups=PAIRS,
                ins=[k_own_d[:]], outs=[k_all_d[:]])
            nc.gpsimd.collective_compute(
                "AllGather", mybir.AluOpType.bypass, replica_groups=PAIRS,
                ins=[v_own_d[:]], outs=[v_all_d[:]])

            kT = [prpool.tile([P, L], MM_DT, tag=f"kT{m}", name=f"kT{m}")
                  for m in range(MC)]
            for m in range(MC):
                for hf in range(2):
                    nc.sync.dma_start(kT[m][:, hf * LQ:(hf + 1) * LQ],
                                      k_all_d[hf, m])
            V = [prpool.tile([P, H * 65], MM_DT, tag=f"V{t}", name=f"V{t}")
                 for t in range(KVC)]
            for t in range(KVC):
                nc.sync.dma_start(V[t][:], v_all_d[t // (KVC // 2),
                                                   t % (KVC // 2)])

            # ---- flash attention per head ----
            xsT2 = [prpool.tile([P, LQ], MM_DT, tag=f"xs{hp}", name=f"xsT2_{hp}")
                    for hp in range(MC)]
            for h in range(H):
                hp, po = h // 2, 64 * (h % 2)
                at = []
                for c in range(KVC):
                    ss = ps.tile([P, 1024], F32, tag="scores", bufs=3)
                    for qh in range(2):
                        nc.tensor.matmul(
                            ss[:, qh * 512:(qh + 1) * 512],
                            kT[hp][po:po + 64, c * P:(c + 1) * P],
                            qT[hp][po:po + 64, qh * 512:(qh + 1) * 512],
                            start=True, stop=True)
                    a = apool.tile([P, 1024], MM_DT, tag="at")
                    nc.scalar.activation(a[:], ss[:], AF.Exp,
                                         bias=mb[:, c:c + 1], scale=0.125)
                    at.append(a)
                xs = [ps.tile([65, 512], F32, tag="proj", name=f"xs_h{h}_{qh}")
                      for qh in range(2)]
                for c in range(KVC):
                    for qh in range(2):
                        nc.tensor.matmul(
                            xs[qh][:], V[c][:, 65 * h:65 * h + 65],
                            at[c][:, qh * 512:(qh + 1) * 512],
                            start=c == 0, stop=c == KVC - 1)
                for qh in range(2):
                    srow = npool.tile([1, 512], F32, tag="srow")
                    nc.vector.tensor_copy(srow[:], xs[qh][64:65, :])
                    rec = npool.tile([1, 512], F32, tag="rec")
                    nc.vector.reciprocal_approx_fast(rec[:], srow[:])
                    bc = npool.tile([64, 512], F32, tag="bc")
                    nc.gpsimd.partition_broadcast(bc[:], rec[:])
                    nc.vector.tensor_mul(
                        xsT2[hp][po:po + 64, qh * 512:(qh + 1) * 512],
                        xs[qh][0:64, :], bc[:])

            # ---- output projection ----
            for qt in range(QT):
                po_ = ps.tile([P, D], F32, tag="proj")
                for hp in range(MC):
                    nc.tensor.matmul(po_[:], xsT2[hp][:, qt * P:(qt + 1) * P],
                                     wo[hp][:, :], start=hp == 0, stop=hp == MC - 1)
                osb = opool.tile([P, D], F32, tag="osb")
                nc.vector.tensor_add(osb[:], po_[:], bo_bc[:])
                nc.sync.dma_start(out_d[qt * P:(qt + 1) * P, :], osb[:])

    nc.compile()
    return nc


def _host_inputs(query, key, value, mask, Wq, bq, Wk, bk, Wv, bv, Wo, bo):
    """Build the 8 per-core input maps (all rank-dependence lives here)."""
    f32 = np.float32
    wq_ = np.ascontiguousarray(Wq).astype(MM_NP)
    wk_ = np.ascontiguousarray(Wk).astype(MM_NP)
    wv_ = np.ascontiguousarray(Wv).astype(MM_NP)
    wo_ = np.ascontiguousarray(Wo).astype(MM_NP)
    bq_ = np.ascontiguousarray(bq.astype(f32).reshape(MC, P).T)
    bk_ = np.ascontiguousarray(bk.astype(f32).reshape(MC, P).T)
    bv_ = bv.astype(MM_NP).reshape(1, D)
    bo_ = bo.astype(f32).reshape(1, D)
    in_maps = []
    for c in range(N_CORES):
        b, half = c // 2, c % 2
        sl = slice(half * LQ, (half + 1) * LQ)
        xqT = np.ascontiguousarray(query[b, sl, :].T).astype(MM_NP)
        xkT = np.ascontiguousarray(key[b, sl, :].T).astype(MM_NP)
        xvT = np.ascontiguousarray(value[b, sl, :].T).astype(MM_NP)
        mbias = np.where(mask[b] == 0, MASK_BIAS, f32(0.0)).astype(f32)
        mb_ = np.ascontiguousarray(mbias.reshape(KVC, P).T)
        in_maps.append({
            "xqT": xqT, "xkT": xkT, "xvT": xvT,
            "wq": wq_, "wk": wk_, "wv": wv_, "wo": wo_,
            "bq": bq_, "bk": bk_, "bv": bv_, "bo": bo_, "mb": mb_,
        })
    return in_maps


def kernel(query, key, value, mask, Wq, bq, Wk, bk, Wv, bv, Wo, bo):
    if "nc" not in _cache:
        _cache["nc"] = _build()
    nc = _cache["nc"]
    in_maps = _host_inputs(query, key, value, mask,
                           Wq, bq, Wk, bk, Wv, bv, Wo, bo)
    res = run_bass_kernel_spmd(nc, in_maps, list(range(N_CORES))).results
    out = np.empty((B, L, D), np.float32)
    for c in range(N_CORES):
        b, half = c // 2, c % 2
        out[b, half * LQ:(half + 1) * LQ, :] = res[c]["out"]
    return out



# revision 11
# speedup vs baseline: 1.8861x; 1.8861x over previous
"""Multi-head attention (B=4, L=2048, D=512, H=8) on 8 Trainium2 cores.

Sharding: core c handles batch b = c//2, query rows [(c%2)*1024, +1024).
The key-padding mask knocks out ~half of all kv positions, so the host
compresses K/V to the unmasked positions only (padded to a multiple of
128); each core projects the full compressed K/V for its batch (no
collectives).  All matmuls are bf16 (fp8 attn was tried: its ~2.4% rms
quantization error lands right at the 2e-2 tolerance).

Device layouts (per core):
  xqT (512, 1024), xkT/xvT (512, KVP)  inputs, dmodel on partitions
  qT/kT (128, LQ|KVP) x4               projections kept transposed; head h
                                       in dmodel-chunk h//2 at part 64*(h%2)
  V (128, 520) per kv chunk            V natural layout; head h at cols
                                       [65h,65h+64), col 65h+64 = ones
  at (128, 1024) bf16                  exp(attn) for one kv chunk
  ss (128kv, 1024q) PSUM               scores; exp+mask+scale in one ACT op
  xs (65, 512) PSUM                    attn@V accum, row 64 = softmax denom
"""
import math

import numpy as np
import ml_dtypes

import concourse.bacc as bacc
import concourse.bass as bass
import concourse.mybir as mybir
import concourse.tile as tile
from concourse.bass_utils import run_bass_kernel_spmd

F32 = mybir.dt.float32
BF16 = mybir.dt.bfloat16
AF = mybir.ActivationFunctionType

B, L, D = 4, 2048, 512
H, DK = 8, 64
N_CORES = 8
LQ = L // 2            # query rows per core
P = 128
MC = D // P            # 4 dmodel chunks
MASK_BIAS = np.float32(-1e30)
EXP_SHIFT = np.float32(0.0)

_cache = {}


def _build(kvpc):
    """kvpc = number of 128-wide kv chunks after mask compression."""
    kvp = kvpc * P

    nc = bacc.Bacc("TRN2", target_bir_lowering=False, debug=False,
                   num_devices=N_CORES)

    xqT_d = nc.dram_tensor("xqT", [D, LQ], BF16, kind="ExternalInput").ap()
    xkT_d = nc.dram_tensor("xkT", [D, kvp], BF16, kind="ExternalInput").ap()
    xvT_d = nc.dram_tensor("xvT", [D, kvp], BF16, kind="ExternalInput").ap()
    wq_d = nc.dram_tensor("wq", [D, D], BF16, kind="ExternalInput").ap()
    wk_d = nc.dram_tensor("wk", [D, D], BF16, kind="ExternalInput").ap()
    wv_d = nc.dram_tensor("wv", [D, D], BF16, kind="ExternalInput").ap()
    wo_d = nc.dram_tensor("wo", [D, D], BF16, kind="ExternalInput").ap()
    bq_d = nc.dram_tensor("bq", [P, MC], F32, kind="ExternalInput").ap()
    bk_d = nc.dram_tensor("bk", [P, MC], F32, kind="ExternalInput").ap()
    bv_d = nc.dram_tensor("bv", [1, D], F32, kind="ExternalInput").ap()
    bo_d = nc.dram_tensor("bo", [1, D], F32, kind="ExternalInput").ap()
    mb_d = nc.dram_tensor("mb", [P, kvpc], F32, kind="ExternalInput").ap()
    out_d = nc.dram_tensor("out", [LQ, D], F32, kind="ExternalOutput").ap()

    kblocks = []
    off = 0
    while off < kvp:
        sz = min(512, kvp - off)
        kblocks.append((off, sz))
        off += sz

    with tile.TileContext(nc) as tc:
        with tc.tile_pool(name="const", bufs=1) as cpool, \
             tc.tile_pool(name="xin", bufs=1) as xpool, \
             tc.tile_pool(name="proj", bufs=1) as prpool, \
             tc.tile_pool(name="attn", bufs=2) as apool, \
             tc.tile_pool(name="norm", bufs=4) as npool, \
             tc.tile_pool(name="outp", bufs=3) as opool, \
             tc.tile_pool(name="ps", bufs=2, space="PSUM") as ps:

            def load_chunks(pool, ap2d, nm):
                out = []
                for kc in range(MC):
                    t = pool.tile([P, ap2d.shape[1]], ap2d.dtype,
                                  tag=f"{nm}{kc}", name=f"{nm}{kc}")
                    nc.sync.dma_start(t[:], ap2d[kc * P:(kc + 1) * P, :])
                    out.append(t)
                return out

            # interleave weight/input chunk loads in first-use order
            wq = load_chunks(cpool, wq_d, "wq")
            xqT = load_chunks(xpool, xqT_d, "xq")
            bq = cpool.tile_from(bq_d)
            wk = load_chunks(cpool, wk_d, "wk")
            xkT = load_chunks(xpool, xkT_d, "xk")
            bk = cpool.tile_from(bk_d)
            mb = cpool.tile_from(mb_d)
            wv = load_chunks(cpool, wv_d, "wv")
            xvT = load_chunks(xpool, xvT_d, "xv")
            bv = cpool.tile_from(bv_d)
            wo = load_chunks(cpool, wo_d, "wo")
            bo = cpool.tile_from(bo_d)
            bv_bc = cpool.tile([P, D], F32)
            nc.gpsimd.partition_broadcast(bv_bc[:], bv[:])
            bo_bc = cpool.tile([P, D], F32)
            nc.gpsimd.partition_broadcast(bo_bc[:], bo[:])

            qT = [prpool.tile([P, LQ], BF16, tag=f"qT{m}", name=f"qT{m}")
                  for m in range(MC)]
            kT = [prpool.tile([P, kvp], BF16, tag=f"kT{m}", name=f"kT{m}")
                  for m in range(MC)]
            V = [prpool.tile([P, H * 65], BF16, tag=f"V{t}", name=f"V{t}")
                 for t in range(kvpc)]
            xsT2 = [prpool.tile([P, LQ], BF16, tag=f"xs{hp}",
                                name=f"xsT2_{hp}") for hp in range(MC)]

            def qproj(m):
                for off, sz in ((0, 512), (512, 512)):
                    pp = ps.tile([P, 512], F32, tag="proj")
                    for kc in range(MC):
                        nc.tensor.matmul(
                            pp[:, 0:sz], wq[kc][:, m * P:(m + 1) * P],
                            xqT[kc][:, off:off + sz],
                            start=kc == 0, stop=kc == MC - 1)
                    nc.vector.tensor_scalar_add(qT[m][:, off:off + sz],
                                                pp[:, 0:sz], bq[:, m:m + 1])

            def kproj(m):
                for off, sz in kblocks:
                    pp = ps.tile([P, 512], F32, tag="proj")
                    for kc in range(MC):
                        nc.tensor.matmul(
                            pp[:, 0:sz], wk[kc][:, m * P:(m + 1) * P],
                            xkT[kc][:, off:off + sz],
                            start=kc == 0, stop=kc == MC - 1)
                    nc.vector.tensor_scalar_add(kT[m][:, off:off + sz],
                                                pp[:, 0:sz], bk[:, m:m + 1])

            def vproj(t):
                pv = ps.tile([P, 512], F32, tag="proj")
                for kc in range(MC):
                    nc.tensor.matmul(pv[:], xvT[kc][:, t * P:(t + 1) * P],
                                     wv[kc][:, :], start=kc == 0,
                                     stop=kc == MC - 1)
                v8 = V[t].rearrange("p (g d) -> p g d", d=65)
                nc.vector.tensor_add(v8[:, :, 0:64],
                                     pv.rearrange("p (g d) -> p g d", d=64),
                                     bv_bc.rearrange("p (g d) -> p g d", d=64))
                nc.vector.memset(v8[:, :, 64:65], 1.0)

            def score_chunk(h, c, at):
                hp, po = h // 2, 64 * (h % 2)
                ss = ps.tile([P, 1024], F32, tag="scores")
                for qh in range(2):
                    nc.tensor.matmul(
                        ss[:, qh * 512:(qh + 1) * 512],
                        kT[hp][po:po + 64, c * P:(c + 1) * P],
                        qT[hp][po:po + 64, qh * 512:(qh + 1) * 512],
                        start=True, stop=True)
                nc.scalar.activation(at[c][:], ss[:], AF.Exp,
                                     bias=mb[:, c:c + 1], scale=0.125)

            def attnv_units(h, at):
                """Yield emission thunks for attn@V + normalize of head h."""
                hp, po = h // 2, 64 * (h % 2)
                xs = [None, None]

                def mk_mm(qh, c):
                    def emit():
                        if c == 0:
                            xs[qh] = ps.tile([65, 512], F32, tag="xs",
                                             name=f"xs_h{h}_{qh}")
                        nc.tensor.matmul(
                            xs[qh][:], V[c][:, 65 * h:65 * h + 65],
                            at[c][:, qh * 512:(qh + 1) * 512],
                            start=c == 0, stop=c == kvpc - 1)
                    return emit

                def mk_norm(qh):
                    def emit():
                        srow = npool.tile([1, 512], F32, tag="srow")
                        nc.vector.tensor_copy(srow[:], xs[qh][64:65, :])
                        rec = npool.tile([1, 512], F32, tag="rec")
                        nc.vector.reciprocal_approx_fast(rec[:], srow[:])
                        bc = npool.tile([64, 512], F32, tag="bc")
                        nc.gpsimd.partition_broadcast(bc[:], rec[:])
                        nc.vector.tensor_mul(
                            xsT2[hp][po:po + 64, qh * 512:(qh + 1) * 512],
                            xs[qh][0:64, :], bc[:])
                    return emit

                for qh in range(2):
                    for c in range(kvpc):
                        yield mk_mm(qh, c)
                    yield mk_norm(qh)

            def new_at(h):
                return [apool.tile([P, 1024], BF16, tag=f"at{c}",
                                   name=f"at_h{h}_{c}")
                        for c in range(kvpc)]

            # ---- emission schedule ----
            qproj(0)
            kproj(0)
            at_prev = new_at(0)
            for c in range(kvpc):
                score_chunk(0, c, at_prev)
            for t in range(kvpc):
                vproj(t)
            qproj(1)
            kproj(1)

            for h in range(1, H):
                if h == 4:
                    qproj(2)
                    kproj(2)
                elif h == 6:
                    qproj(3)
                    kproj(3)
                at_cur = new_at(h)
                units = list(attnv_units(h - 1, at_prev))
                ui = 0
                for c in range(kvpc):
                    score_chunk(h, c, at_cur)
                    # ~2 attnV/normalize units per score chunk keeps the
                    # tensor queue from head-of-line blocking on exp
                    take = 2 if c < kvpc - 1 else len(units) - ui
                    for _ in range(take):
                        if ui < len(units):
                            units[ui]()
                            ui += 1
                at_prev = at_cur
            for emit in attnv_units(H - 1, at_prev):
                emit()

            # ---- output projection ----
            for qt in range(LQ // P):
                po_ = ps.tile([P, D], F32, tag="proj")
                for hp in range(MC):
                    nc.tensor.matmul(po_[:], xsT2[hp][:, qt * P:(qt + 1) * P],
                                     wo[hp][:, :], start=hp == 0,
                                     stop=hp == MC - 1)
                osb = opool.tile([P, D], F32, tag="osb")
                nc.vector.tensor_add(osb[:], po_[:], bo_bc[:])
                nc.sync.dma_start(out_d[qt * P:(qt + 1) * P, :], osb[:])

    nc.compile()
    return nc


def _host_inputs(query, key, value, mask, Wq, bq, Wk, bk, Wv, bv, Wo, bo):
    """Build the 8 per-core input maps; returns (in_maps, kvpc)."""
    f32, bf16 = np.float32, ml_dtypes.bfloat16
    idxs = [np.flatnonzero(mask[b]) for b in range(B)]
    cnts = [len(ix) for ix in idxs]
    kvpc = max(2, math.ceil(max(cnts) / P))
    kvp = kvpc * P

    wq_ = np.ascontiguousarray(Wq).astype(bf16)
    wk_ = np.ascontiguousarray(Wk).astype(bf16)
    wv_ = np.ascontiguousarray(Wv).astype(bf16)
    wo_ = np.ascontiguousarray(Wo).astype(bf16)
    bq_ = np.ascontiguousarray(bq.astype(f32).reshape(MC, P).T)
    bk_ = np.ascontiguousarray(bk.astype(f32).reshape(MC, P).T)
    bv_ = bv.astype(f32).reshape(1, D)
    bo_ = bo.astype(f32).reshape(1, D)

    per_batch = []
    for b in range(B):
        cnt = cnts[b]
        xk = np.zeros((kvp, D), f32)
        xv = np.zeros((kvp, D), f32)
        xk[:cnt] = key[b][idxs[b]]
        xv[:cnt] = value[b][idxs[b]]
        xkT = np.ascontiguousarray(xk.T).astype(bf16)
        xvT = np.ascontiguousarray(xv.T).astype(bf16)
        mbias = np.where(np.arange(kvp) < cnt, EXP_SHIFT, MASK_BIAS)
        mb_ = np.ascontiguousarray(mbias.astype(f32).reshape(kvpc, P).T)
        per_batch.append((xkT, xvT, mb_))

    in_maps = []
    for c in range(N_CORES):
        b, half = c // 2, c % 2
        sl = slice(half * LQ, (half + 1) * LQ)
        xqT = np.ascontiguousarray(query[b, sl, :].T).astype(bf16)
        xkT, xvT, mb_ = per_batch[b]
        in_maps.append({
            "xqT": xqT, "xkT": xkT, "xvT": xvT,
            "wq": wq_, "wk": wk_, "wv": wv_, "wo": wo_,
            "bq": bq_, "bk": bk_, "bv": bv_, "bo": bo_, "mb": mb_,
        })
    return in_maps, kvpc


def kernel(query, key, value, mask, Wq, bq, Wk, bk, Wv, bv, Wo, bo):
    in_maps, kvpc = _host_inputs(query, key, value, mask,
                                 Wq, bq, Wk, bk, Wv, bv, Wo, bo)
    if kvpc not in _cache:
        _cache[kvpc] = _build(kvpc)
    nc = _cache[kvpc]
    res = run_bass_kernel_spmd(nc, in_maps, list(range(N_CORES))).results
    out = np.empty((B, L, D), np.float32)
    for c in range(N_CORES):
        b, half = c // 2, c % 2
        out[b, half * LQ:(half + 1) * LQ, :] = res[c]["out"]
    return out


# revision 16
# speedup vs baseline: 1.9224x; 1.0193x over previous
"""Multi-head attention (B=4, L=2048, D=512, H=8) on 8 Trainium2 cores.

Sharding: core c handles batch b = c//2, query rows [(c%2)*1024, +1024).
The key-padding mask knocks out ~half of all kv positions, so the host
compresses K/V to the unmasked positions only (padded to a multiple of
128); each core projects the full compressed K/V for its batch (no
collectives).  All matmuls are bf16 (fp8 attn was tried: its ~2.4% rms
quantization error lands right at the 2e-2 tolerance).

Device layouts (per core):
  xqT (512, 1024), xkT/xvT (512, KVP)  inputs, dmodel on partitions
  qT/kT (128, LQ|KVP) x4               projections kept transposed; head h
                                       in dmodel-chunk h//2 at part 64*(h%2)
  V (128, 520) per kv chunk            V natural layout; head h at cols
                                       [65h,65h+64), col 65h+64 = ones
  at (128, 1024) bf16                  exp(attn) for one kv chunk
  ss (128kv, 1024q) PSUM               scores; exp+mask+scale in one ACT op
  xs (65, 512) PSUM                    attn@V accum, row 64 = softmax denom
"""
import math

import numpy as np
import ml_dtypes

import concourse.bacc as bacc
import concourse.bass as bass
import concourse.mybir as mybir
import concourse.tile as tile
from concourse.bass_utils import run_bass_kernel_spmd

F32 = mybir.dt.float32
BF16 = mybir.dt.bfloat16
AF = mybir.ActivationFunctionType

B, L, D = 4, 2048, 512
H, DK = 8, 64
N_CORES = 8
LQ = L // 2            # query rows per core
P = 128
MC = D // P            # 4 dmodel chunks
MASK_BIAS = np.float32(-1e30)
EXP_SHIFT = np.float32(0.0)

_cache = {}


def _build(kvpc):
    """kvpc = number of 128-wide kv chunks after mask compression."""
    kvp = kvpc * P

    nc = bacc.Bacc("TRN2", target_bir_lowering=False, debug=False,
                   num_devices=N_CORES)

    xqT_d = nc.dram_tensor("xqT", [D, LQ], BF16, kind="ExternalInput").ap()
    xkT_d = nc.dram_tensor("xkT", [D, kvp], BF16, kind="ExternalInput").ap()
    xvT_d = nc.dram_tensor("xvT", [D, kvp], BF16, kind="ExternalInput").ap()
    # wq/wk are m-major on host: [m, p, kc*128+mc] so the m=0 slice loads
    # first and projections can start ~2.5us in
    wq_d = nc.dram_tensor("wq", [MC, P, D], BF16, kind="ExternalInput").ap()
    wk_d = nc.dram_tensor("wk", [MC, P, D], BF16, kind="ExternalInput").ap()
    wv_d = nc.dram_tensor("wv", [D, D], BF16, kind="ExternalInput").ap()
    wo_d = nc.dram_tensor("wo", [D, D], BF16, kind="ExternalInput").ap()
    bq_d = nc.dram_tensor("bq", [P, MC], F32, kind="ExternalInput").ap()
    bk_d = nc.dram_tensor("bk", [P, MC], F32, kind="ExternalInput").ap()
    bv_d = nc.dram_tensor("bv", [1, D], F32, kind="ExternalInput").ap()
    bo_d = nc.dram_tensor("bo", [1, D], F32, kind="ExternalInput").ap()
    mb_d = nc.dram_tensor("mb", [P, kvpc], F32, kind="ExternalInput").ap()
    out_d = nc.dram_tensor("out", [LQ, D], F32, kind="ExternalOutput").ap()

    kblocks = []
    off = 0
    while off < kvp:
        sz = min(512, kvp - off)
        kblocks.append((off, sz))
        off += sz

    with tile.TileContext(nc) as tc:
        with tc.tile_pool(name="const", bufs=1) as cpool, \
             tc.tile_pool(name="xin", bufs=1) as xpool, \
             tc.tile_pool(name="proj", bufs=1) as prpool, \
             tc.tile_pool(name="attn", bufs=2) as apool, \
             tc.tile_pool(name="norm", bufs=4) as npool, \
             tc.tile_pool(name="outp", bufs=3) as opool, \
             tc.tile_pool(name="ps", bufs=2, space="PSUM") as ps:

            def chunk_tiles(pool, cols, dtype, nm):
                return [pool.tile([P, cols], dtype, tag=f"{nm}{kc}",
                                  name=f"{nm}{kc}") for kc in range(MC)]

            # tiles declared up front; DMAs emitted in first-use order below
            wqm = [cpool.tile([P, D], BF16, tag=f"wqm{m}", name=f"wqm{m}")
                   for m in range(MC)]
            wkm = [cpool.tile([P, D], BF16, tag=f"wkm{m}", name=f"wkm{m}")
                   for m in range(MC)]
            xqT = chunk_tiles(xpool, LQ, BF16, "xq")
            xkT = chunk_tiles(xpool, kvp, BF16, "xk")
            xvT = chunk_tiles(xpool, kvp, BF16, "xv")
            wv = chunk_tiles(cpool, D, BF16, "wv")
            wo = chunk_tiles(cpool, D, BF16, "wo")

            nc.sync.dma_start(wqm[0][:], wq_d[0])
            for kc in range(MC):
                nc.sync.dma_start(xqT[kc][:, 0:512],
                                  xqT_d[kc * P:(kc + 1) * P, 0:512])
            bq = cpool.tile_from(bq_d)
            bk = cpool.tile_from(bk_d)
            mb = cpool.tile_from(mb_d)
            nc.sync.dma_start(wkm[0][:], wk_d[0])
            for off, sz in kblocks:
                for kc in range(MC):
                    nc.sync.dma_start(xkT[kc][:, off:off + sz],
                                      xkT_d[kc * P:(kc + 1) * P, off:off + sz])
            for kc in range(MC):
                nc.sync.dma_start(xqT[kc][:, 512:1024],
                                  xqT_d[kc * P:(kc + 1) * P, 512:1024])
            bv = cpool.tile_from(bv_d)
            bo = cpool.tile_from(bo_d)
            bv_bc = cpool.tile([P, D], F32)
            nc.gpsimd.partition_broadcast(bv_bc[:], bv[:])
            bo_bc = cpool.tile([P, D], F32)
            nc.gpsimd.partition_broadcast(bo_bc[:], bo[:])
            for kc in range(MC):
                nc.sync.dma_start(wv[kc][:], wv_d[kc * P:(kc + 1) * P, :])
                nc.sync.dma_start(xvT[kc][:], xvT_d[kc * P:(kc + 1) * P, :])
            for m in range(1, MC):
                nc.sync.dma_start(wqm[m][:], wq_d[m])
                nc.sync.dma_start(wkm[m][:], wk_d[m])
            for kc in range(MC):
                nc.sync.dma_start(wo[kc][:], wo_d[kc * P:(kc + 1) * P, :])

            qT = [prpool.tile([P, LQ], BF16, tag=f"qT{m}", name=f"qT{m}")
                  for m in range(MC)]
            kT = [prpool.tile([P, kvp], BF16, tag=f"kT{m}", name=f"kT{m}")
                  for m in range(MC)]
            V = [prpool.tile([P, H * 65], BF16, tag=f"V{t}", name=f"V{t}")
                 for t in range(kvpc)]
            xsT2 = [prpool.tile([P, LQ], BF16, tag=f"xs{hp}",
                                name=f"xsT2_{hp}") for hp in range(MC)]

            def qproj(m):
                wv4 = wqm[m].rearrange("p (kc mc) -> p kc mc", mc=P)
                for off, sz in ((0, 512), (512, 512)):
                    pp = ps.tile([P, 512], F32, tag="proj")
                    for kc in range(MC):
                        nc.tensor.matmul(
                            pp[:, 0:sz], wv4[:, kc, :],
                            xqT[kc][:, off:off + sz],
                            start=kc == 0, stop=kc == MC - 1)
                    nc.vector.tensor_scalar_add(qT[m][:, off:off + sz],
                                                pp[:, 0:sz], bq[:, m:m + 1])

            def kproj(m):
                wv4 = wkm[m].rearrange("p (kc mc) -> p kc mc", mc=P)
                for off, sz in kblocks:
                    pp = ps.tile([P, 512], F32, tag="proj")
                    for kc in range(MC):
                        nc.tensor.matmul(
                            pp[:, 0:sz], wv4[:, kc, :],
                            xkT[kc][:, off:off + sz],
                            start=kc == 0, stop=kc == MC - 1)
                    nc.vector.tensor_scalar_add(kT[m][:, off:off + sz],
                                                pp[:, 0:sz], bk[:, m:m + 1])

            def vproj(t):
                pv = ps.tile([P, 512], F32, tag="proj")
                for kc in range(MC):
                    nc.tensor.matmul(pv[:], xvT[kc][:, t * P:(t + 1) * P],
                                     wv[kc][:, :], start=kc == 0,
                                     stop=kc == MC - 1)
                v8 = V[t].rearrange("p (g d) -> p g d", d=65)
                nc.vector.tensor_add(v8[:, :, 0:64],
                                     pv.rearrange("p (g d) -> p g d", d=64),
                                     bv_bc.rearrange("p (g d) -> p g d", d=64))
                nc.vector.memset(v8[:, :, 64:65], 1.0)

            def score_chunk(h, c, at):
                hp, po = h // 2, 64 * (h % 2)
                ss = ps.tile([P, 1024], F32, tag="scores")
                for qh in range(2):
                    nc.tensor.matmul(
                        ss[:, qh * 512:(qh + 1) * 512],
                        kT[hp][po:po + 64, c * P:(c + 1) * P],
                        qT[hp][po:po + 64, qh * 512:(qh + 1) * 512],
                        start=True, stop=True)
                nc.scalar.activation(at[c][:], ss[:], AF.Exp,
                                     bias=mb[:, c:c + 1], scale=0.125)

            def attnv_units(h, at):
                """Yield emission thunks for attn@V + normalize of head h."""
                hp, po = h // 2, 64 * (h % 2)
                xs = [None, None]

                def mk_mm(qh, c):
                    def emit():
                        if c == 0:
                            xs[qh] = ps.tile([65, 512], F32, tag="xs",
                                             name=f"xs_h{h}_{qh}")
                        nc.tensor.matmul(
                            xs[qh][:], V[c][:, 65 * h:65 * h + 65],
                            at[c][:, qh * 512:(qh + 1) * 512],
                            start=c == 0, stop=c == kvpc - 1)
                    return emit

                def mk_norm(qh):
                    def emit():
                        srow = npool.tile([1, 512], F32, tag="srow")
                        nc.vector.tensor_copy(srow[:], xs[qh][64:65, :])
                        rec = npool.tile([1, 512], F32, tag="rec")
                        nc.vector.reciprocal_approx_fast(rec[:], srow[:])
                        bc = npool.tile([64, 512], F32, tag="bc")
                        nc.gpsimd.partition_broadcast(bc[:], rec[:])
                        nc.vector.tensor_mul(
                            xsT2[hp][po:po + 64, qh * 512:(qh + 1) * 512],
                            xs[qh][0:64, :], bc[:])
                    return emit

                for qh in range(2):
                    for c in range(kvpc):
                        yield mk_mm(qh, c)
                    yield mk_norm(qh)

            def new_at(h):
                return [apool.tile([P, 1024], BF16, tag=f"at{c}",
                                   name=f"at_h{h}_{c}")
                        for c in range(kvpc)]

            # ---- emission schedule ----
            qproj(0)
            kproj(0)
            at_prev = new_at(0)
            for c in range(kvpc):
                score_chunk(0, c, at_prev)
            for t in range(kvpc):
                vproj(t)
            qproj(1)
            kproj(1)

            for h in range(1, H):
                if h == 4:
                    qproj(2)
                    kproj(2)
                elif h == 6:
                    qproj(3)
                    kproj(3)
                at_cur = new_at(h)
                units = list(attnv_units(h - 1, at_prev))
                ui = 0
                for c in range(kvpc):
                    score_chunk(h, c, at_cur)
                    # ~2 attnV/normalize units per score chunk keeps the
                    # tensor queue from head-of-line blocking on exp
                    take = 2 if c < kvpc - 1 else len(units) - ui
                    for _ in range(take):
                        if ui < len(units):
                            units[ui]()
                            ui += 1
                at_prev = at_cur

            def oproj(qt):
                po_ = ps.tile([P, D], F32, tag="proj")
                for hp in range(MC):
                    nc.tensor.matmul(po_[:], xsT2[hp][:, qt * P:(qt + 1) * P],
                                     wo[hp][:, :], start=hp == 0,
                                     stop=hp == MC - 1)
                osb = opool.tile([P, D], F32, tag="osb")
                nc.vector.tensor_add(osb[:], po_[:], bo_bc[:])
                nc.sync.dma_start(out_d[qt * P:(qt + 1) * P, :], osb[:])

            # last head: output projection of each query-half starts as soon
            # as that half's softmax normalize lands
            last = list(attnv_units(H - 1, at_prev))
            nqh = len(last) // 2
            for emit in last[:nqh]:
                emit()
            for qt in range(4):
                oproj(qt)
            for emit in last[nqh:]:
                emit()
            for qt in range(4, 8):
                oproj(qt)

    nc.compile()
    return nc


def _host_inputs(query, key, value, mask, Wq, bq, Wk, bk, Wv, bv, Wo, bo):
    """Build the 8 per-core input maps; returns (in_maps, kvpc)."""
    f32, bf16 = np.float32, ml_dtypes.bfloat16
    idxs = [np.flatnonzero(mask[b]) for b in range(B)]
    cnts = [len(ix) for ix in idxs]
    kvpc = max(2, math.ceil(max(cnts) / P))
    kvp = kvpc * P

    def m_major(W):  # [m, p, kc*128+mc]: W[kc*128+p, m*128+mc]
        return np.ascontiguousarray(
            W.reshape(MC, P, MC, P).transpose(2, 1, 0, 3).reshape(MC, P, MC * P)
        ).astype(bf16)

    wq_ = m_major(np.asarray(Wq))
    wk_ = m_major(np.asarray(Wk))
    wv_ = np.ascontiguousarray(Wv).astype(bf16)
    wo_ = np.ascontiguousarray(Wo).astype(bf16)
    bq_ = np.ascontiguousarray(bq.astype(f32).reshape(MC, P).T)
    bk_ = np.ascontiguousarray(bk.astype(f32).reshape(MC, P).T)
    bv_ = bv.astype(f32).reshape(1, D)
    bo_ = bo.astype(f32).reshape(1, D)

    per_batch = []
    for b in range(B):
        cnt = cnts[b]
        xk = np.zeros((kvp, D), f32)
        xv = np.zeros((kvp, D), f32)
        xk[:cnt] = key[b][idxs[b]]
        xv[:cnt] = value[b][idxs[b]]
        xkT = np.ascontiguousarray(xk.T).astype(bf16)
        xvT = np.ascontiguousarray(xv.T).astype(bf16)
        mbias = np.where(np.arange(kvp) < cnt, EXP_SHIFT, MASK_BIAS)
        mb_ = np.ascontiguousarray(mbias.astype(f32).reshape(kvpc, P).T)
        per_batch.append((xkT, xvT, mb_))

    in_maps = []
    for c in range(N_CORES):
        b, half = c // 2, c % 2
        sl = slice(half * LQ, (half + 1) * LQ)
        xqT = np.ascontiguousarray(query[b, sl, :].T).astype(bf16)
        xkT, xvT, mb_ = per_batch[b]
        in_maps.append({
            "xqT": xqT, "xkT": xkT, "xvT": xvT,
            "wq": wq_, "wk": wk_, "wv": wv_, "wo": wo_,
            "bq": bq_, "bk": bk_, "bv": bv_, "bo": bo_, "mb": mb_,
        })
    return in_maps, kvpc


def kernel(query, key, value, mask, Wq, bq, Wk, bk, Wv, bv, Wo, bo):
    in_maps, kvpc = _host_inputs(query, key, value, mask,
                                 Wq, bq, Wk, bk, Wv, bv, Wo, bo)
    if kvpc not in _cache:
        _cache[kvpc] = _build(kvpc)
    nc = _cache[kvpc]
    res = run_bass_kernel_spmd(nc, in_maps, list(range(N_CORES))).results
    out = np.empty((B, L, D), np.float32)
    for c in range(N_CORES):
        b, half = c // 2, c % 2
        out[b, half * LQ:(half + 1) * LQ, :] = res[c]["out"]
    return out
